# revision 5
# baseline (speedup 1.0000x reference)
"""Distortion-regularization loss on Trainium2 (8 NeuronCores, SPMD).

Math: the reference loss collapses to a single quadratic form
    loss = mean_n( w_n^T A w_n ),   A = |u_i - u_j| + diag(ds)/3   (32x32 const)
         = <A, W^T W> / N_RAYS
so each core only needs the Gram matrix of its ray shard:
    Gram_c = W_c^T W_c   (32x32, accumulated on the TensorEngine in fp32 PSUM)
and the scalar partial  <A/N, Gram_c>.  The host sums the 8 partials.

Per-core kernel (data parallel over rays, per the sharding hint):
  - stream the [259200, 32] f32 shard in 46 tiles of [128, 1408] (44 rays per
    partition per tile) + one [128, 32] tail tile
  - convert each tile to bf16 on DVE (rel. error of the final mean ~5e-6,
    validated off-line; fp32 matmul is 4x slower per PE row)
  - for each 128-column window (4 ray-groups) issue one matmul with
    lhsT = rhs = window: the [128, 128] PSUM accumulator picks up the four
    useful 32x32 diagonal Gram blocks at (32q, 32q); off-diagonal blocks are
    cross-ray garbage that a block-diagonal weight matrix masks out later
  - contract PSUM with the block-diagonal A/N const (DVE tensor_tensor_reduce)
    and finish the cross-partition sum with a ones-vector matmul -> [1,1]
"""

import numpy as np

NEAR = 0.2
FAR = 1000.0
BINS = 32
N_RAYS = 2073600
N_CORES = 8
N_SHARD = N_RAYS // N_CORES        # 259200 rays per core
P = 128
K = 44                             # rays per partition per main tile
F = K * BINS                       # 1408 f32 per partition
N_TILES = 46                       # 46*128*44 = 259072 rays
LEFT = N_SHARD - N_TILES * P * K   # 128 leftover rays
WINDOWS = K // 4                   # 11 N=128 matmul windows per tile

# set by test.py to capture a neuron-profile trace; harness leaves it False
TRACE = False
TRACE_TMPDIR = None
LAST_RESULTS = None


def _a_matrix() -> np.ndarray:
    eps = float(np.finfo(np.float32).eps)
    t = np.linspace(NEAR + eps, FAR, BINS + 1, dtype=np.float32)
    s = ((1.0 / t) - (1.0 / (NEAR + eps))) / ((1.0 / FAR) - (1.0 / (NEAR + eps)))
    s = s.astype(np.float32)
    us = ((s[1:] + s[:-1]) * 0.5).astype(np.float32)
    dus = np.abs(us[:, None] - us[None, :]).astype(np.float32)
    ds = (s[1:] - s[:-1]).astype(np.float32)
    return (dus + np.diag(ds) / 3.0).astype(np.float32)


_COMPILED = None


def _build():
    import concourse.bacc as bacc
    import concourse.mybir as mybir
    from concourse import tile

    nc = bacc.Bacc("TRN2", debug=False)
    f32 = mybir.dt.float32
    bf16 = mybir.dt.bfloat16

    ws = nc.dram_tensor("ws", [N_SHARD, BINS], f32, kind="ExternalInput")
    out = nc.dram_tensor("out", [1, 1], f32, kind="ExternalOutput")

    a = _a_matrix() / np.float32(N_RAYS)
    bigw = np.zeros((P, P), np.float32)
    for q in range(4):
        bigw[32 * q:32 * q + 32, 32 * q:32 * q + 32] = a
    bigw_d = nc.inline_tensor(bigw, name="bigw")

    with tile.TileContext(nc) as tc:
        with (
            tc.tile_pool(name="const", bufs=1) as const_pool,
            tc.tile_pool(name="io", bufs=3) as io_pool,
            tc.tile_pool(name="bf", bufs=3) as bf_pool,
            tc.tile_pool(name="fin", bufs=1) as fin_pool,
            tc.tile_pool(name="psum", bufs=1, space="PSUM") as psum_pool,
        ):
            bigw_s = const_pool.tile([P, P], f32)
            nc.sync.dma_start(bigw_s[:], bigw_d[:])
            ones_s = const_pool.tile([P, 1], f32)
            nc.vector.memset(ones_s[:], 1.0)

            gram_ps = psum_pool.tile([P, P], f32)

            main = ws[0:N_TILES * P * K, :].rearrange(
                "(t p k) b -> t p (k b)", p=P, k=K
            )
            mm = 0
            n_mm = N_TILES * WINDOWS
            for t in range(N_TILES):
                ftile = io_pool.tile([P, F], f32)
                nc.sync.dma_start(ftile[:], main[t])
                btile = bf_pool.tile([P, F], bf16)
                nc.vector.tensor_copy(btile[:], ftile[:])
                for w in range(WINDOWS):
                    sl = btile[:, w * 128:(w + 1) * 128]
                    nc.tensor.matmul(
                        gram_ps[:], sl, sl, start=(mm == 0), stop=(mm == n_mm - 1)
                    )
                    mm += 1

            # leftover 128 rays: own PSUM tile (separate accumulation group);
            # its [32, 32] Gram block is folded in during the final reduction
            left_ps = psum_pool.tile([32, 32], f32, tag="left")
            lf = io_pool.tile([P, BINS], f32, tag="lf")
            nc.sync.dma_start(lf[:], ws[N_TILES * P * K:N_SHARD, :])
            lb = bf_pool.tile([P, BINS], bf16, tag="lb")
            nc.vector.tensor_copy(lb[:], lf[:])
            nc.tensor.matmul(left_ps[:], lb[:], lb[:], start=True, stop=True)

            # tensor_tensor_reduce hits a runtime failure on HW via this
            # compile path (probe.py stage 3) — use mul + reduce instead
            prod_s = fin_pool.tile([P, P], f32)
            acc_s = fin_pool.tile([P, 1], f32)
            nc.vector.tensor_mul(prod_s[:], gram_ps[:], bigw_s[:])
            nc.vector.reduce_sum(acc_s[:], prod_s[:], axis=mybir.AxisListType.X)
            lprod_s = fin_pool.tile([32, 32], f32)
            lacc_s = fin_pool.tile([32, 1], f32)
            nc.vector.tensor_mul(lprod_s[:], left_ps[:], bigw_s[0:32, 0:32])
            nc.vector.reduce_sum(lacc_s[:], lprod_s[:], axis=mybir.AxisListType.X)
            res_ps = psum_pool.tile([1, 1], f32, tag="res")
            nc.tensor.matmul(res_ps[:], acc_s[:], ones_s[:], start=True, stop=False)
            nc.tensor.matmul(
                res_ps[:], lacc_s[:], ones_s[0:32, :], start=False, stop=True
            )
            out_s = fin_pool.tile([1, 1], f32)
            nc.vector.tensor_copy(out_s[:], res_ps[:])
            nc.sync.dma_start(out[:], out_s[:])

    nc.compile()
    return nc


def kernel(ws: np.ndarray) -> np.ndarray:
    from concourse.bass_utils import run_bass_kernel_spmd

    global _COMPILED, LAST_RESULTS
    if _COMPILED is None:
        _COMPILED = _build()
    nc = _COMPILED

    ws = np.ascontiguousarray(np.asarray(ws), dtype=np.float32)
    assert ws.shape == (N_RAYS, BINS), ws.shape
    shards = ws.reshape(N_CORES, N_SHARD, BINS)
    in_maps = [{"ws": shards[c]} for c in range(N_CORES)]
    res = run_bass_kernel_spmd(
        nc, in_maps, list(range(N_CORES)), trace=TRACE, tmpdir=TRACE_TMPDIR
    )
    LAST_RESULTS = res
    total = np.float64(0.0)
    for c in range(N_CORES):
        total += np.float64(res.results[c]["out"][0, 0])
    return np.array(total, dtype=np.float32)


# revision 8
# speedup vs baseline: 1.1353x; 1.1353x over previous
"""Distortion-regularization loss on Trainium2 (8 NeuronCores, SPMD).

Math: the reference loss collapses to a single quadratic form
    loss = mean_n( w_n^T A w_n ),   A = |u_i - u_j| + diag(ds)/3   (32x32 const)
         = <A, W^T W> / N_RAYS
so each core only needs the Gram matrix of its ray shard:
    Gram_c = W_c^T W_c   (32x32, accumulated on the TensorEngine in fp32 PSUM)
and the scalar partial  <A/N, Gram_c>.  The host sums the 8 partials.

Per-core kernel (data parallel over rays, per the sharding hint):
  - stream the [259200, 32] f32 shard in 46 tiles of [128, 1408] (44 rays per
    partition per tile) + one [128, 32] tail tile
  - convert each tile to bf16 on DVE (rel. error of the final mean ~5e-6,
    validated off-line; fp32 matmul is 4x slower per PE row)
  - for each 128-column window (4 ray-groups) issue one matmul with
    lhsT = rhs = window: the [128, 128] PSUM accumulator picks up the four
    useful 32x32 diagonal Gram blocks at (32q, 32q); off-diagonal blocks are
    cross-ray garbage that a block-diagonal weight matrix masks out later
  - contract PSUM with the block-diagonal A/N const (DVE tensor_tensor_reduce)
    and finish the cross-partition sum with a ones-vector matmul -> [1,1]
"""

import numpy as np

NEAR = 0.2
FAR = 1000.0
BINS = 32
N_RAYS = 2073600
N_CORES = 8
N_SHARD = N_RAYS // N_CORES        # 259200 rays per core
P = 128
K = 88                             # rays per partition per main tile
F = K * BINS                       # f32 elements per partition per tile
N_TILES = 23                       # main tiles; N_TILES*128*K + 128 == N_SHARD
LEFT = N_SHARD - N_TILES * P * K   # 128 leftover rays
WINDOWS = K // 4                   # N=128 matmul windows per tile
IO_BUFS = 4
BF_BUFS = 3
ALT_DMA = True                     # alternate sync/scalar HWDGE rings

# set by test.py to capture a neuron-profile trace; harness leaves it False
TRACE = False
TRACE_TMPDIR = None
LAST_RESULTS = None


def _a_matrix() -> np.ndarray:
    eps = float(np.finfo(np.float32).eps)
    t = np.linspace(NEAR + eps, FAR, BINS + 1, dtype=np.float32)
    s = ((1.0 / t) - (1.0 / (NEAR + eps))) / ((1.0 / FAR) - (1.0 / (NEAR + eps)))
    s = s.astype(np.float32)
    us = ((s[1:] + s[:-1]) * 0.5).astype(np.float32)
    dus = np.abs(us[:, None] - us[None, :]).astype(np.float32)
    ds = (s[1:] - s[:-1]).astype(np.float32)
    return (dus + np.diag(ds) / 3.0).astype(np.float32)


_COMPILED = None


def _build():
    import concourse.bacc as bacc
    import concourse.mybir as mybir
    from concourse import tile

    nc = bacc.Bacc("TRN2", debug=False)
    f32 = mybir.dt.float32
    bf16 = mybir.dt.bfloat16

    ws = nc.dram_tensor("ws", [N_SHARD, BINS], f32, kind="ExternalInput")
    out = nc.dram_tensor("out", [1, 1], f32, kind="ExternalOutput")

    a = _a_matrix() / np.float32(N_RAYS)
    bigw = np.zeros((P, P), np.float32)
    for q in range(4):
        bigw[32 * q:32 * q + 32, 32 * q:32 * q + 32] = a
    bigw_d = nc.inline_tensor(bigw, name="bigw")

    with tile.TileContext(nc) as tc:
        with (
            tc.tile_pool(name="const", bufs=1) as const_pool,
            tc.tile_pool(name="io", bufs=IO_BUFS) as io_pool,
            tc.tile_pool(name="bf", bufs=BF_BUFS) as bf_pool,
            tc.tile_pool(name="fin", bufs=1) as fin_pool,
            tc.tile_pool(name="psum", bufs=1, space="PSUM") as psum_pool,
        ):
            bigw_s = const_pool.tile([P, P], f32)
            nc.sync.dma_start(bigw_s[:], bigw_d[:])
            ones_s = const_pool.tile([P, 1], f32)
            nc.vector.memset(ones_s[:], 1.0)

            gram_ps = psum_pool.tile([P, P], f32)

            main = ws[0:N_TILES * P * K, :].rearrange(
                "(t p k) b -> t p (k b)", p=P, k=K
            )
            mm = 0
            n_mm = N_TILES * WINDOWS
            for t in range(N_TILES):
                ftile = io_pool.tile([P, F], f32)
                dma_eng = nc.scalar if (ALT_DMA and t % 2) else nc.sync
                dma_eng.dma_start(ftile[:], main[t])
                btile = bf_pool.tile([P, F], bf16)
                nc.vector.tensor_copy(btile[:], ftile[:])
                for w in range(WINDOWS):
                    sl = btile[:, w * 128:(w + 1) * 128]
                    nc.tensor.matmul(
                        gram_ps[:], sl, sl, start=(mm == 0), stop=(mm == n_mm - 1)
                    )
                    mm += 1

            # leftover 128 rays: own PSUM tile (separate accumulation group);
            # its [32, 32] Gram block is folded in during the final reduction
            left_ps = psum_pool.tile([32, 32], f32, tag="left")
            lf = io_pool.tile([P, BINS], f32, tag="lf")
            nc.sync.dma_start(lf[:], ws[N_TILES * P * K:N_SHARD, :])
            lb = bf_pool.tile([P, BINS], bf16, tag="lb")
            nc.vector.tensor_copy(lb[:], lf[:])
            nc.tensor.matmul(left_ps[:], lb[:], lb[:], start=True, stop=True)

            # tensor_tensor_reduce hits a runtime failure on HW via this
            # compile path (probe.py stage 3) — use mul + reduce instead
            prod_s = fin_pool.tile([P, P], f32)
            acc_s = fin_pool.tile([P, 1], f32)
            nc.vector.tensor_mul(prod_s[:], gram_ps[:], bigw_s[:])
            nc.vector.reduce_sum(acc_s[:], prod_s[:], axis=mybir.AxisListType.X)
            lprod_s = fin_pool.tile([32, 32], f32)
            lacc_s = fin_pool.tile([32, 1], f32)
            nc.vector.tensor_mul(lprod_s[:], left_ps[:], bigw_s[0:32, 0:32])
            nc.vector.reduce_sum(lacc_s[:], lprod_s[:], axis=mybir.AxisListType.X)
            res_ps = psum_pool.tile([1, 1], f32, tag="res")
            nc.tensor.matmul(res_ps[:], acc_s[:], ones_s[:], start=True, stop=False)
            nc.tensor.matmul(
                res_ps[:], lacc_s[:], ones_s[0:32, :], start=False, stop=True
            )
            out_s = fin_pool.tile([1, 1], f32)
            nc.vector.tensor_copy(out_s[:], res_ps[:])
            nc.sync.dma_start(out[:], out_s[:])

    nc.compile()
    return nc


def kernel(ws: np.ndarray) -> np.ndarray:
    from concourse.bass_utils import run_bass_kernel_spmd

    global _COMPILED, LAST_RESULTS
    if _COMPILED is None:
        _COMPILED = _build()
    nc = _COMPILED

    ws = np.ascontiguousarray(np.asarray(ws), dtype=np.float32)
    assert ws.shape == (N_RAYS, BINS), ws.shape
    shards = ws.reshape(N_CORES, N_SHARD, BINS)
    in_maps = [{"ws": shards[c]} for c in range(N_CORES)]
    res = run_bass_kernel_spmd(
        nc, in_maps, list(range(N_CORES)), trace=TRACE, tmpdir=TRACE_TMPDIR
    )
    LAST_RESULTS = res
    total = np.float64(0.0)
    for c in range(N_CORES):
        total += np.float64(res.results[c]["out"][0, 0])
    return np.array(total, dtype=np.float32)


# revision 10
# speedup vs baseline: 1.3158x; 1.1590x over previous
"""Distortion-regularization loss on Trainium2 (8 NeuronCores, SPMD).

Math: the reference loss collapses to a single quadratic form
    loss = mean_n( w_n^T A w_n ),   A = |u_i - u_j| + diag(ds)/3   (32x32 const)
         = <A, W^T W> / N_RAYS
so each core only needs the Gram matrix of its ray shard:
    Gram_c = W_c^T W_c   (32x32, accumulated on the TensorEngine in fp32 PSUM)
and the scalar partial  <A/N, Gram_c>.  The host sums the 8 partials.

Per-core kernel (data parallel over rays, per the sharding hint):
  - stream the [259200, 32] f32 shard in 46 tiles of [128, 1408] (44 rays per
    partition per tile) + one [128, 32] tail tile
  - convert each tile to bf16 on DVE (rel. error of the final mean ~5e-6,
    validated off-line; fp32 matmul is 4x slower per PE row)
  - for each 128-column window (4 ray-groups) issue one matmul with
    lhsT = rhs = window: the [128, 128] PSUM accumulator picks up the four
    useful 32x32 diagonal Gram blocks at (32q, 32q); off-diagonal blocks are
    cross-ray garbage that a block-diagonal weight matrix masks out later
  - contract PSUM with the block-diagonal A/N const (DVE tensor_tensor_reduce)
    and finish the cross-partition sum with a ones-vector matmul -> [1,1]
"""

import numpy as np

NEAR = 0.2
FAR = 1000.0
BINS = 32
N_RAYS = 2073600
N_CORES = 8
N_SHARD = N_RAYS // N_CORES        # 259200 rays per core
P = 128
# per-tile rays-per-partition: big 1.4MB tiles for DMA efficiency, tapering
# at the end so the post-DMA cast+matmul tail chain is short. Each K must be
# divisible by 4 (whole N=128 matmul windows); sum(K)*128 + 128 == N_SHARD.
TILE_KS = [88] * 22 + [56, 24, 8]
assert sum(TILE_KS) * P + P == N_SHARD
IO_BUFS = 6
BF_BUFS = 4
ALT_DMA = True                     # alternate sync/scalar HWDGE rings

# set by test.py to capture a neuron-profile trace; harness leaves it False
TRACE = False
TRACE_TMPDIR = None
LAST_RESULTS = None


def _a_matrix() -> np.ndarray:
    eps = float(np.finfo(np.float32).eps)
    t = np.linspace(NEAR + eps, FAR, BINS + 1, dtype=np.float32)
    s = ((1.0 / t) - (1.0 / (NEAR + eps))) / ((1.0 / FAR) - (1.0 / (NEAR + eps)))
    s = s.astype(np.float32)
    us = ((s[1:] + s[:-1]) * 0.5).astype(np.float32)
    dus = np.abs(us[:, None] - us[None, :]).astype(np.float32)
    ds = (s[1:] - s[:-1]).astype(np.float32)
    return (dus + np.diag(ds) / 3.0).astype(np.float32)


_COMPILED = None


def _build():
    import concourse.bacc as bacc
    import concourse.mybir as mybir
    from concourse import tile

    nc = bacc.Bacc("TRN2", debug=False)
    f32 = mybir.dt.float32
    bf16 = mybir.dt.bfloat16

    ws = nc.dram_tensor("ws", [N_SHARD, BINS], f32, kind="ExternalInput")
    out = nc.dram_tensor("out", [1, 1], f32, kind="ExternalOutput")

    a = _a_matrix() / np.float32(N_RAYS)
    bigw = np.zeros((P, P), np.float32)
    for q in range(4):
        bigw[32 * q:32 * q + 32, 32 * q:32 * q + 32] = a
    bigw_d = nc.inline_tensor(bigw, name="bigw")

    with tile.TileContext(nc) as tc:
        with (
            tc.tile_pool(name="const", bufs=1) as const_pool,
            tc.tile_pool(name="io", bufs=IO_BUFS) as io_pool,
            tc.tile_pool(name="bf", bufs=BF_BUFS) as bf_pool,
            tc.tile_pool(name="fin", bufs=1) as fin_pool,
            tc.tile_pool(name="psum", bufs=1, space="PSUM") as psum_pool,
        ):
            bigw_s = const_pool.tile([P, P], f32)
            nc.sync.dma_start(bigw_s[:], bigw_d[:])
            ones_s = const_pool.tile([P, 1], f32)
            nc.vector.memset(ones_s[:], 1.0)

            gram_ps = psum_pool.tile([P, P], f32)

            mm = 0
            n_mm = sum(TILE_KS) // 4
            ray0 = 0
            for t, kt in enumerate(TILE_KS):
                ft = kt * BINS
                view = ws[ray0:ray0 + P * kt, :].rearrange(
                    "(p k) b -> p (k b)", p=P, k=kt
                )
                ray0 += P * kt
                ftile = io_pool.tile([P, ft], f32, tag="ftile")
                dma_eng = nc.scalar if (ALT_DMA and t % 2) else nc.sync
                dma_eng.dma_start(ftile[:], view)
                btile = bf_pool.tile([P, ft], bf16, tag="btile")
                nc.vector.tensor_copy(btile[:], ftile[:])
                for w in range(kt // 4):
                    sl = btile[:, w * 128:(w + 1) * 128]
                    nc.tensor.matmul(
                        gram_ps[:], sl, sl, start=(mm == 0), stop=(mm == n_mm - 1)
                    )
                    mm += 1

            # leftover 128 rays: own PSUM tile (separate accumulation group);
            # its [32, 32] Gram block is folded in during the final reduction
            left_ps = psum_pool.tile([32, 32], f32, tag="left")
            lf = io_pool.tile([P, BINS], f32, tag="lf")
            nc.sync.dma_start(lf[:], ws[ray0:N_SHARD, :])
            lb = bf_pool.tile([P, BINS], bf16, tag="lb")
            nc.vector.tensor_copy(lb[:], lf[:])
            nc.tensor.matmul(left_ps[:], lb[:], lb[:], start=True, stop=True)

            # tensor_tensor_reduce hits a runtime failure on HW via this
            # compile path (probe.py stage 3) — use mul + reduce instead
            prod_s = fin_pool.tile([P, P], f32)
            acc_s = fin_pool.tile([P, 1], f32)
            nc.vector.tensor_mul(prod_s[:], gram_ps[:], bigw_s[:])
            nc.vector.reduce_sum(acc_s[:], prod_s[:], axis=mybir.AxisListType.X)
            lprod_s = fin_pool.tile([32, 32], f32)
            lacc_s = fin_pool.tile([32, 1], f32)
            nc.vector.tensor_mul(lprod_s[:], left_ps[:], bigw_s[0:32, 0:32])
            nc.vector.reduce_sum(lacc_s[:], lprod_s[:], axis=mybir.AxisListType.X)
            res_ps = psum_pool.tile([1, 1], f32, tag="res")
            nc.tensor.matmul(res_ps[:], acc_s[:], ones_s[:], start=True, stop=False)
            nc.tensor.matmul(
                res_ps[:], lacc_s[:], ones_s[0:32, :], start=False, stop=True
            )
            out_s = fin_pool.tile([1, 1], f32)
            nc.vector.tensor_copy(out_s[:], res_ps[:])
            nc.sync.dma_start(out[:], out_s[:])

    nc.compile()
    return nc


def kernel(ws: np.ndarray) -> np.ndarray:
    from concourse.bass_utils import run_bass_kernel_spmd

    global _COMPILED, LAST_RESULTS
    if _COMPILED is None:
        _COMPILED = _build()
    nc = _COMPILED

    ws = np.ascontiguousarray(np.asarray(ws), dtype=np.float32)
    assert ws.shape == (N_RAYS, BINS), ws.shape
    shards = ws.reshape(N_CORES, N_SHARD, BINS)
    in_maps = [{"ws": shards[c]} for c in range(N_CORES)]
    res = run_bass_kernel_spmd(
        nc, in_maps, list(range(N_CORES)), trace=TRACE, tmpdir=TRACE_TMPDIR
    )
    LAST_RESULTS = res
    total = np.float64(0.0)
    for c in range(N_CORES):
        total += np.float64(res.results[c]["out"][0, 0])
    return np.array(total, dtype=np.float32)
